# revision 44
# baseline (speedup 1.0000x reference)
"""AttentiveFP forward on 8 Trainium2 NeuronCores (Bass/Tile).

Nodes sharded at graph boundaries (batch is sorted), so every graph's nodes
live on one core and the attentive readout is fully local (no readout
collective, unpaired local gathers). Edges sharded by dst-owner core;
per-core nodes sorted by in-degree with a round-robin slot structure so
segment softmax/sum become dense PSUM matmul accumulation.

Per layer: SBUF-resident gather index table (loaded once, shared by all
layers), paired-256/512B dma_gather of fp16 rows (idx = table_row>>1 fits
int16; parity select on DVE), softmax logit chain via the identity
exp(leaky(x)) = max(exp(x), exp(0.01 x)) so the ACT engine never switches
function tables, denominators fused into the alpha mask multiply via
scalar_tensor_tensor accum_out, identity-matmul numerator reduction in
PSUM, fp16 GRU (2 fused 128-row gate matmuls), and a chunk-major two-piece
AllGather of the fp16 row table fired mid-build so transfers overlap the
row construction. Readout gathers from a local table; per-block batched
attention chains over all timestep slots.
"""
import numpy as np
from contextlib import ExitStack

import concourse.bass as bass
import concourse.tile as tile
from concourse import bacc, mybir
from concourse.bass_utils import run_bass_kernel_spmd
from concourse.masks import make_identity

F32 = mybir.dt.float32
F16 = mybir.dt.float16
I16 = mybir.dt.int16
AF = mybir.ActivationFunctionType
OP = mybir.AluOpType

NCORE = 8
N, E, B = 50000, 500000, 2048
H = 64
NS = 0.01
GPC = B // NCORE
GBLK = (GPC + 127) // 128
GPAD = GBLK * 128
MAXR0 = 10
MAXR2 = 20
MAXRG = 8
CH = 512


def _calls_for(R, maxr):
    calls, base, bases = [], 0, []
    for r in R:
        bases.append(base)
        calls.append([(r0, min(r0 + maxr, int(r))) for r0 in range(0, int(r), maxr)])
        base += int(r)
    return calls, bases, base


def _wrap_into(gidx, arr, col0):
    n = arr.shape[0]
    blk = arr.reshape(n // 16, 16).T
    gidx[:16, col0:col0 + n // 16] = blk
    gidx[16:128, col0:col0 + n // 16] = np.tile(blk, (7, 1))


def build_plan(edge_index, batch):
    src = edge_index[0].astype(np.int64)
    dst = edge_index[1].astype(np.int64)

    gsize = np.bincount(batch, minlength=B)
    gstart = np.concatenate([[0], np.cumsum(gsize)])
    bounds = np.array([gstart[c * GPC] for c in range(NCORE)] + [N], np.int64)
    npcs = [int(bounds[c + 1] - bounds[c]) for c in range(NCORE)]
    NBLK = (max(npcs) + 127) // 128
    NPAD = NBLK * 128
    TROWS = NCORE * NPAD

    owner = np.searchsorted(bounds[1:], dst, side="right")

    perms, degs_sorted, grp_starts, egrp = [], [], [], []
    sortpos = np.zeros(N, np.int64)
    for c in range(NCORE):
        n0 = bounds[c]
        emask = np.nonzero(owner == c)[0]
        deg = np.bincount(dst[emask] - n0, minlength=npcs[c])
        order = np.argsort(-deg, kind="stable")
        perms.append(order)
        sortpos[n0 + order] = np.arange(npcs[c])
        dsorted = deg[order]
        degs_sorted.append(dsorted)
        eorder = np.argsort(sortpos[dst[emask]], kind="stable")
        egrp.append(emask[eorder])
        grp_starts.append(np.concatenate([[0], np.cumsum(dsorted)]))
    node_owner = np.searchsorted(bounds[1:], np.arange(N), side="right")
    HQ = (NBLK // 2) * 64            # pair-rows per core in collective chunk A
    q = sortpos >> 1
    prow = np.where(q < HQ, node_owner * HQ + q,
                    NCORE * HQ + node_owner * (NPAD // 2 - HQ) + (q - HQ))
    parbit = sortpos & 1

    R = np.ones(NBLK, np.int64)
    for b in range(NBLK):
        for c in range(NCORE):
            d = degs_sorted[c][b * 128:(b + 1) * 128]
            if len(d):
                R[b] = max(R[b], int(d[0]))
    calls, bases, NCH = _calls_for(R, MAXR0)
    calls2, _, _ = _calls_for(R, MAXR2)
    S = NCH * 128

    gperms, gss = [], []
    for c in range(NCORE):
        gs = gsize[c * GPC:(c + 1) * GPC]
        gorder = np.argsort(-gs, kind="stable")
        gperms.append(gorder)
        gss.append(gs[gorder])
    RG = np.ones(GBLK, np.int64)
    for b in range(GBLK):
        for c in range(NCORE):
            d = gss[c][b * 128:(b + 1) * 128]
            if len(d):
                RG[b] = max(RG[b], int(d[0]))
    gcalls, gbases, GCH = _calls_for(RG, MAXRG)
    SR = GCH * 128

    cores = []
    lanes = np.arange(128)
    for c in range(NCORE):
        gidx = np.zeros((128, S // 16), np.int16)
        par = np.zeros((128, NCH, 1), np.float16)
        msk = np.zeros((128, NCH, 1), np.float16)
        esel = np.full(NCH * 128, -1, np.int64)
        ds = degs_sorted[c]
        gst = grp_starts[c]
        eg = egrp[c]
        npc = npcs[c]
        for b in range(NBLK):
            for (r0, r1) in calls[b]:
                ia = np.zeros((r1 - r0) * 128, np.int64)
                for r in range(r0, r1):
                    ch = bases[b] + r
                    p = b * 128 + lanes
                    pc = np.minimum(p, npc - 1)
                    ok = (p < npc) & (r < ds[pc])
                    eids = gst[pc] + r
                    ge = np.where(ok, eg[np.where(ok, np.minimum(eids, len(eg) - 1), 0)], -1)
                    esel[ch * 128 + lanes] = ge
                    sj = src[np.maximum(ge, 0)]
                    ia[(r - r0) * 128 + lanes] = np.where(ok, prow[sj], 0)
                    par[:, ch, 0] = np.where(ok, parbit[sj], 0).astype(np.float16)
                    msk[:, ch, 0] = ok.astype(np.float32)
                _wrap_into(gidx, ia, (bases[b] + r0) * 8)
        cores.append(dict(gidx=gidx, par=par, msk=msk, esel=esel))

    # readout plan: fully local (graph-aligned sharding), unpaired row idx
    rcores = []
    for c in range(NCORE):
        rgidx = np.zeros((128, SR // 16), np.int16)
        rmsk = np.zeros((128, GCH, 1), np.float32)
        rmsk16 = np.zeros((128, GCH, 1), np.float16)
        gs = gss[c]
        gp = gperms[c]
        for b in range(GBLK):
            for (r0, r1) in gcalls[b]:
                ia = np.zeros((r1 - r0) * 128, np.int64)
                for r in range(r0, r1):
                    ch = gbases[b] + r
                    p = b * 128 + lanes
                    pc = np.minimum(p, GPC - 1)
                    ok = (p < GPC) & (r < gs[pc])
                    g = c * GPC + gp[pc]
                    node = np.where(ok, gstart[g] + r, bounds[c])
                    ia[(r - r0) * 128 + lanes] = sortpos[node]
                    rmsk[:, ch, 0] = ok.astype(np.float32)
                    rmsk16[:, ch, 0] = ok.astype(np.float16)
                _wrap_into(rgidx, ia, (gbases[b] + r0) * 8)
        rcores.append(dict(rgidx=rgidx, rmsk=rmsk, rmsk16=rmsk16))

    return dict(R=R, calls=calls, calls2=calls2, bases=bases, NCH=NCH, S=S,
                RG=RG, gcalls=gcalls, gbases=gbases, GCH=GCH, SR=SR,
                NBLK=NBLK, NPAD=NPAD, TROWS=TROWS, bounds=bounds, npcs=npcs,
                cores=cores, rcores=rcores, perms=perms, gperms=gperms)


def build_nc(plan):
    R, calls, bases, NCH, S = plan["R"], plan["calls"], plan["bases"], plan["NCH"], plan["S"]
    calls2 = plan["calls2"]
    RG, gcalls, gbases, GCH, SR = plan["RG"], plan["gcalls"], plan["gbases"], plan["GCH"], plan["SR"]
    NBLK, NPAD, TROWS = plan["NBLK"], plan["NPAD"], plan["TROWS"]

    nc = bacc.Bacc("TRN2", target_bir_lowering=False, debug=False,
                   num_devices=NCORE, num_swdge_queues=4)

    def din(name, shape, dt=F32):
        return nc.dram_tensor(name, shape, dt, kind="ExternalInput")

    xT_in = din("xT_in", [H, NPAD], F16)
    gidx_in = din("gidx", [128, S // 16], I16)
    par_in = din("par", [128, NCH, 1], F16)
    msk_in = din("msk", [128, NCH, 1], F16)
    eaT_in = din("eaT", [16, S], F16)
    rgidx_in = din("rgidx", [128, SR // 16], I16)
    rmsk_in = din("rmsk", [128, GCH, 1], F32)
    rmsk16_in = din("rmsk16", [128, GCH, 1], F16)
    lin1T = din("lin1T", [H, H], F16); lin1_b = din("lin1_b", [H, 1])
    w1aT = din("w1aT", [H, H], F16); w1bT = din("w1bT", [16, H], F16)
    attl_rep = din("attl_rep", [128, 1, H], F16)
    attr_rep = din("attr_rep", [128, H], F16)
    g2T = din("g2T", [H, H], F16); gate_b = din("gate_b", [H, 1])
    atomT = din("atomT", [H, 2, H], F16)
    asrc_rep = din("asrc_rep", [128, 2, H], F16)
    adst_rep = din("adst_rep", [128, 2, H], F16)
    atom_b = din("atom_b", [H, 2])
    molT = din("molT", [H, H], F16)
    mol_lin = din("mol_lin", [H, H])
    matt_src_rep = din("matt_src_rep", [128, 1, H], F16)
    matt_dst = din("matt_dst", [H, 1])
    mol_b = din("mol_b", [H, 1])
    gruW = din("gruW", [128, 4, 128], F16)   # [K, widx, r|z]
    gruN = din("gruN", [128, 4, 128], F16)   # [K, widx, nx|nh] zero-padded
    gbx_rz = din("gbx_rz", [64, 8])
    gbh_rz = din("gbh_rz", [64, 8])
    gbx_n = din("gbx_n", [H, 4])
    gbh_n = din("gbh_n", [H, 4])
    lin2T = din("lin2T", [H, H], F16); lin2_b = din("lin2_b", [H, 1])
    lng_rep = din("lng_rep", [128, H]); lnb_rep = din("lnb_rep", [128, H])
    h1T = din("h1T", [H, H], F16); h1_b = din("h1_b", [H, 1])
    h2T = din("h2T", [H, H], F16); h2_b = din("h2_b", [H, 1])
    h3T = din("h3T", [H, 1], F16); h3_b = din("h3_b", [1, 1])

    y_out = nc.dram_tensor("y_out", [1, GPAD], F32, kind="ExternalOutput")

    cins, couts = [], []
    for l in range(3):
        cw = 128 if l == 0 else 64
        cins.append(nc.dram_tensor(f"cin{l}", [NPAD, cw], F16))
        couts.append(nc.dram_tensor(f"cout{l}", [TROWS // 2, 2 * cw], F16,
                                    addr_space="Shared"))
    routT = nc.dram_tensor("routT", [NPAD, 128], F16)

    ctx = ExitStack()
    ctx2 = nc.allow_low_precision(reason="fp16 edge tables/messages")
    ctx2.__enter__()
    with tile.TileContext(nc) as tc:
        cpool = ctx.enter_context(tc.tile_pool(name="const", bufs=1))
        wpool = ctx.enter_context(tc.tile_pool(name="wts", bufs=1))
        big = ctx.enter_context(tc.tile_pool(name="big", bufs=1))
        stkp = ctx.enter_context(tc.tile_pool(name="stkp", bufs=2))
        xsp = ctx.enter_context(tc.tile_pool(name="xsp", bufs=2))
        gp = ctx.enter_context(tc.tile_pool(name="gath", bufs=4))
        grp_ = ctx.enter_context(tc.tile_pool(name="gathr", bufs=3))
        sp = ctx.enter_context(tc.tile_pool(name="scr", bufs=2))
        sp3 = ctx.enter_context(tc.tile_pool(name="scr3", bufs=3))
        s1 = ctx.enter_context(tc.tile_pool(name="scr1", bufs=1))
        rowp = ctx.enter_context(tc.tile_pool(name="rows", bufs=5))
        pp = ctx.enter_context(tc.tile_pool(name="ps", bufs=2, space="PSUM"))
        rp = ctx.enter_context(tc.tile_pool(name="psr", bufs=2, space="PSUM"))
        zp = ctx.enter_context(tc.tile_pool(name="psz", bufs=1, space="PSUM"))

        id32 = cpool.tile([128, 128], F32)
        make_identity(nc, id32[:])
        id16 = cpool.tile([128, 128], F16)
        nc.vector.tensor_copy(id16[:], id32[:])

        _ldctr = [0]

        def load(t, shape, dt=F32):
            s = wpool.tile(shape, dt, tag=f"w_{t.name}")
            eng = nc.sync if _ldctr[0] % 2 == 0 else nc.scalar
            _ldctr[0] += 1
            eng.dma_start(s[:], t[:])
            return s

        gidx_s = load(gidx_in, [128, S // 16], I16)
        rgidx_s = load(rgidx_in, [128, SR // 16], I16)
        par_s = load(par_in, [128, NCH, 1], F16)
        msk_s = load(msk_in, [128, NCH, 1], F16)
        rmsk_s = load(rmsk_in, [128, GCH, 1], F32)
        rmsk16_s = load(rmsk16_in, [128, GCH, 1], F16)
        lin1T_s = load(lin1T, [H, H], F16); lin1b_s = load(lin1_b, [H, 1])
        w1aT_s = load(w1aT, [H, H], F16); w1bT_s = load(w1bT, [16, H], F16)
        attl_s = load(attl_rep, [128, 1, H], F16)
        attr_s = load(attr_rep, [128, H], F16)
        g2T_s = load(g2T, [H, H], F16); gateb_s = load(gate_b, [H, 1])
        atomT_s = load(atomT, [H, 2, H], F16)
        asrc_s = load(asrc_rep, [128, 2, H], F16)
        adst_s = load(adst_rep, [128, 2, H], F16)
        atomb_s = load(atom_b, [H, 2])
        molT_s = load(molT, [H, H], F16); mol_lin_s = load(mol_lin, [H, H])
        msrc_s = load(matt_src_rep, [128, 1, H], F16)
        mdst_s = load(matt_dst, [H, 1])
        molb_s = load(mol_b, [H, 1])
        gruW_s = load(gruW, [128, 4, 128], F16)
        gruN_s = load(gruN, [128, 4, 128], F16)
        gbxrz_s = load(gbx_rz, [64, 8])
        gbhrz_s = load(gbh_rz, [64, 8])
        gbxn_s = load(gbx_n, [H, 4])
        gbhn_s = load(gbh_n, [H, 4])
        lin2T_s = load(lin2T, [H, H], F16); lin2b_s = load(lin2_b, [H, 1])
        lng_s = load(lng_rep, [128, H]); lnb_s = load(lnb_rep, [128, H])
        h1T_s = load(h1T, [H, H], F16); h1b_s = load(h1_b, [H, 1])
        h2T_s = load(h2T, [H, H], F16); h2b_s = load(h2_b, [H, 1])
        h3T_s = load(h3T, [H, 1], F16); h3b_s = load(h3_b, [1, 1])

        def fm_mm(out_ap, lhsT, rhs, ncols, func=None, bias=0.0):
            M = lhsT.shape[-1]
            for c0 in range(0, ncols, CH):
                w = min(CH, ncols - c0)
                ps = pp.tile([128, CH], F32, tag="mmq")
                nc.tensor.matmul(ps[0:M, :w], lhsT, rhs[:, c0:c0 + w],
                                 start=True, stop=True)
                f = func
                if f is None:
                    f = AF.Copy if isinstance(bias, float) else AF.Identity
                nc.scalar.activation(out_ap[:, c0:c0 + w], ps[0:M, :w], f, bias=bias)

        def transp(in_ap, out_dt, tag):
            a, bdim = in_ap.shape[0], in_ap.shape[-1]
            ps = rp.tile([128, 128], F32, tag="tp")
            if in_ap.dtype == F16:
                pv = ps[:].bitcast(F16)
                nc.tensor.transpose(pv[0:bdim, 0:a], in_ap, id16[0:a, 0:a])
                src = pv[0:bdim, 0:a]
            else:
                nc.tensor.transpose(ps[0:bdim, 0:a], in_ap, id32[0:a, 0:a])
                src = ps[0:bdim, 0:a]
            t = sp.tile([128, 128], out_dt, tag=tag)
            nc.scalar.activation(t[0:bdim, 0:a], src, AF.Copy)
            return t

        def gru(stacked, xsep, widx, out_ap, ncols):
            br = sp.tile([64, 1], F32, tag="brz")
            nc.vector.tensor_add(br[:], gbxrz_s[:, 2 * widx:2 * widx + 1], gbhrz_s[:, 2 * widx:2 * widx + 1])
            bz = sp.tile([64, 1], F32, tag="bz")
            nc.vector.tensor_add(bz[:], gbxrz_s[:, 2 * widx + 1:2 * widx + 2], gbhrz_s[:, 2 * widx + 1:2 * widx + 2])
            for c0 in range(0, ncols, CH):
                w = min(CH, ncols - c0)
                prz = pp.tile([128, CH], F32, tag="mmq")
                nc.tensor.matmul(prz[:, :w], gruW_s[:, widx], stacked[:, c0:c0 + w], start=True, stop=True)
                rz = sp3.tile([64, CH], F16, tag="rz")
                nc.scalar.activation(rz[:, :w], prz[0:64, :w], AF.Sigmoid, bias=br[:])
                zz = sp3.tile([64, CH], F16, tag="zz")
                nc.scalar.activation(zz[:, :w], prz[64:128, :w], AF.Sigmoid, bias=bz[:])
                pn = pp.tile([128, CH], F32, tag="mmq")
                nc.tensor.matmul(pn[:, :w], gruN_s[:, widx], stacked[:, c0:c0 + w], start=True, stop=True)
                hnb = sp3.tile([64, CH], F16, tag="hnb")
                nc.scalar.activation(hnb[:, :w], pn[64:128, :w], AF.Identity, bias=gbhn_s[:, widx:widx + 1])
                t1 = sp3.tile([64, CH], F32, tag="t1")
                nc.vector.tensor_mul(t1[:, :w], rz[:, :w], hnb[:, :w])
                nc.vector.tensor_add(t1[:, :w], t1[:, :w], pn[0:64, :w])
                nn = sp3.tile([64, CH], F16, tag="nn")
                nc.scalar.activation(nn[:, :w], t1[:, :w], AF.Tanh, bias=gbxn_s[:, widx:widx + 1])
                d = sp3.tile([64, CH], F16, tag="dd1")
                nc.vector.tensor_sub(d[:, :w], xsep[:, c0:c0 + w], nn[:, :w])
                nc.vector.tensor_mul(d[:, :w], zz[:, :w], d[:, :w])
                nc.vector.tensor_add(d[:, :w], nn[:, :w], d[:, :w])
                nc.scalar.activation(out_ap[:, c0:c0 + w], d[:, :w], AF.Relu)

        def elu_inplace(buf, bias_ap, ncols, pre_lhsT=None):
            """buf[0:64, :] = elu((pre_lhsT.T @ buf[0:64]) + bias)."""
            for c0 in range(0, ncols, CH):
                w = min(CH, ncols - c0)
                if pre_lhsT is not None:
                    ps = pp.tile([128, CH], F32, tag="mmq")
                    nc.tensor.matmul(ps[0:64, :w], pre_lhsT, buf[0:64, c0:c0 + w], start=True, stop=True)
                    src = ps[0:64, :w]
                else:
                    src = buf[0:64, c0:c0 + w]
                e1 = sp.tile([64, CH], F16, tag="e1")
                nc.scalar.activation(e1[:, :w], src, AF.Exp, bias=bias_ap)
                t1 = sp.tile([64, CH], F16, tag="el1")
                nc.scalar.activation(t1[:, :w], e1[:, :w], AF.Relu, bias=1.0, scale=-1.0)
                t2 = sp.tile([64, CH], F16, tag="el2")
                nc.scalar.activation(t2[:, :w], src, AF.Relu, bias=bias_ap)
                nc.vector.tensor_sub(buf[0:64, c0:c0 + w], t2[:, :w], t1[:, :w])

        qctr = [0]

        def edge_phase(layer, table, dvals, hdst):
            """hdst[0:64, :NPAD] <- normalized aggregation (feature-major)."""
            MX = MAXR0 if layer == 0 else MAXR2
            EW = 256 if layer == 0 else 128
            W = 128 if layer == 0 else 64
            lcalls = calls if layer == 0 else calls2
            dv001 = sp.tile([128, NBLK], F32, tag="dv001")
            nc.vector.tensor_scalar(dv001[:], dvals[:], NS, None, OP.mult)
            for b in range(NBLK):
                pred = rp.tile([128, 64], F32, tag="red")
                dsum = sp.tile([128, 1], F32, tag="dsum")
                nc.vector.memset(dsum[:], 1e-16)
                Rb = int(R[b])
                for (r0, r1) in lcalls[b]:
                    cr = r1 - r0
                    ch0 = bases[b] + r0
                    graw = gp.tile([128, 4096], F16, tag="g")
                    g = graw[:].rearrange("p (a b) -> p a b", b=EW)
                    nc.gpsimd.dma_gather(
                        g[:, 0:cr], table[:], gidx_s[:, ch0 * 8:(ch0 + cr) * 8],
                        cr * 128, cr * 128, EW, elem_step=EW,
                        single_packet=False, queue_num=qctr[0] % 4)
                    qctr[0] += 1
                    row = sp.tile([128, MX, W], F16, tag="row")
                    nc.vector.tensor_sub(row[:, 0:cr], g[:, 0:cr, W:2 * W], g[:, 0:cr, 0:W])
                    nc.vector.tensor_mul(row[:, 0:cr], row[:, 0:cr],
                                         par_s[:, ch0:ch0 + cr].to_broadcast([128, cr, W]))
                    nc.vector.tensor_add(row[:, 0:cr], row[:, 0:cr], g[:, 0:cr, 0:W])
                    lg = sp.tile([128, MX, 1], F32, tag="lg")
                    if layer == 0:
                        ea = s1.tile([16, MAXR0 * 128], F16, tag="ea")
                        eng = nc.sync if b % 2 == 0 else nc.scalar
                        eng.dma_start(ea[:, 0:cr * 128], eaT_in[:, ch0 * 128:(ch0 + cr) * 128])
                        pz = zp.tile([128, 10, 64], F32, tag="z1")
                        for r in range(cr):
                            nc.tensor.matmul(pz[:, r], ea[:, r * 128:(r + 1) * 128],
                                             w1bT_s[:], start=True, stop=True)
                        ev = sp.tile([128, 10, 64], F16, tag="ev")
                        nc.vector.tensor_add(ev[:, 0:cr], row[:, 0:cr, 64:128], pz[:, 0:cr])
                        evn = sp.tile([128, 10, 64], F16, tag="lr")
                        nc.scalar.activation(evn[:, 0:cr], ev[:, 0:cr], AF.Relu, scale=-(1.0 - NS))
                        nc.vector.tensor_add(ev[:, 0:cr], ev[:, 0:cr], evn[:, 0:cr])
                        sv = sp.tile([128, 10, 64], F16, tag="sv0")
                        nc.vector.tensor_mul(sv[:, 0:cr], ev[:, 0:cr],
                                             attl_s[:].to_broadcast([128, cr, 64]))
                        nc.vector.tensor_reduce(lg[:, 0:cr], sv[:, 0:cr], mybir.AxisListType.X, OP.add)
                        lg_ap = lg[:, 0:cr]
                    else:
                        sv = sp.tile([128, MX, 64], F16, tag="sv")
                        nc.vector.tensor_mul(sv[:, 0:cr], row[:, 0:cr, 0:64],
                                             asrc_s[:, layer - 1:layer].to_broadcast([128, cr, 64]))
                        nc.vector.tensor_reduce(lg[:, 0:cr], sv[:, 0:cr], mybir.AxisListType.X, OP.add)
                        lg_ap = lg[:, 0:cr]
                    e1 = sp.tile([128, MX, 1], F32, tag="lgl")
                    nc.scalar.activation(e1[:, 0:cr], lg_ap, AF.Exp, bias=dvals[:, b:b + 1])
                    e2 = sp.tile([128, MX, 1], F32, tag="lgl2")
                    nc.scalar.activation(e2[:, 0:cr], lg_ap, AF.Exp, scale=NS, bias=dv001[:, b:b + 1])
                    nc.vector.tensor_max(e1[:, 0:cr], e1[:, 0:cr], e2[:, 0:cr])
                    p16 = sp.tile([128, MX, 1], F16, tag="p16")
                    dcall = sp.tile([128, 1], F32, tag="dcall")
                    nc.vector.scalar_tensor_tensor(
                        p16[:, 0:cr], e1[:, 0:cr], 1.0, msk_s[:, ch0:ch0 + cr],
                        OP.mult, OP.mult, accum_out=dcall[:])
                    nc.vector.tensor_add(dsum[:], dsum[:], dcall[:])
                    msg = sp.tile([128, MX, 64], F16, tag="msg")
                    nc.vector.tensor_mul(msg[:, 0:cr], row[:, 0:cr, 0:64],
                                         p16[:, 0:cr].to_broadcast([128, cr, 64]))
                    for r in range(cr):
                        nc.tensor.matmul(pred[:, 0:64], id16[:], msg[:, r],
                                         start=(r0 + r == 0), stop=(r0 + r == Rb - 1))
                rec = sp.tile([128, 1], F32, tag="rec")
                nc.vector.reciprocal(rec[:], dsum[:])
                hnm = sp.tile([128, 64], F32, tag="hnm")
                nc.scalar.activation(hnm[:], pred[:, 0:64], AF.Copy, scale=rec[:])
                ps = rp.tile([128, 128], F32, tag="tp")
                nc.tensor.transpose(ps[0:64, 0:128], hnm[:], id32[:])
                nc.scalar.activation(hdst[0:64, b * 128:(b + 1) * 128], ps[0:64, 0:128], AF.Copy)

        def build_rows(cin_t, width, x_fm, lhsT_A, lhsT_B, dst_rep, src_rep, src_col,
                       dv=None, mid_cb=None):
            cview = cin_t[:].rearrange("(b p) e -> b p e", p=128)
            """rows[:, b, 0:64] = (A@x_b).T (A None -> x_b.T); cols 64:128 =
            (B@x_b).T (B "plain" -> x_b.T, None -> skip); aux dots appended."""
            for b in range(NBLK):
                rows_b = rowp.tile([128, 256], F16, tag="rb")
                if x_fm.base_partition() != 0:
                    xb0 = sp.tile([64, 128], F16, tag="xb0")
                    nc.vector.tensor_copy(xb0[:], x_fm[:, b * 128:(b + 1) * 128])
                    xb_ap = xb0[:]
                else:
                    xb_ap = x_fm[:, b * 128:(b + 1) * 128]
                if lhsT_A is not None:
                    ps = pp.tile([128, CH], F32, tag="mmq")
                    nc.tensor.matmul(ps[0:64, 0:128], lhsT_A, xb_ap, start=True, stop=True)
                    xa = sp.tile([64, 128], F16, tag="xb")
                    nc.scalar.activation(xa[:], ps[0:64, 0:128], AF.Copy)
                    src_fm = xa[:]
                else:
                    src_fm = xb_ap
                pst = rp.tile([128, 128], F32, tag="tp")
                pstv = pst[:].bitcast(F16)
                nc.tensor.transpose(pstv[0:128, 0:64], src_fm, id16[0:64, 0:64])
                nc.scalar.activation(rows_b[:, 0:64], pstv[0:128, 0:64], AF.Copy)
                if lhsT_B is not None:
                    if isinstance(lhsT_B, str):
                        tb = transp(xb_ap, F16, "tb")
                    else:
                        ps2 = pp.tile([128, CH], F32, tag="mmq")
                        nc.tensor.matmul(ps2[0:64, 0:128], lhsT_B, xb_ap, start=True, stop=True)
                        xb2 = sp.tile([64, 128], F16, tag="xb")
                        nc.scalar.activation(xb2[:], ps2[0:64, 0:128], AF.Copy)
                        tb = transp(xb2[:], F16, "tb")
                    nc.vector.tensor_copy(rows_b[:, 64:128], tb[:, 0:64])
                if lhsT_B is None and width > 64:
                    nc.vector.memset(rows_b[:, 64:128], 0.0)
                if src_rep is not None:
                    m = sp.tile([128, H], F32, tag="auxm")
                    nc.vector.tensor_mul(m[:], rows_b[:, 0:64], src_rep)
                    nc.vector.tensor_reduce(rows_b[:, src_col:src_col + 1], m[:], mybir.AxisListType.X, OP.add)
                if dst_rep is not None:
                    m2 = sp.tile([128, H], F32, tag="dvm")
                    nc.vector.tensor_mul(m2[:], rows_b[:, 0:64], dst_rep)
                    nc.vector.tensor_reduce(dv[:, b:b + 1], m2[:], mybir.AxisListType.X, OP.add)

                eng = nc.sync if b % 2 == 0 else nc.scalar
                eng.dma_start(cview[b], rows_b[:, 0:width])
                if mid_cb is not None and b == NBLK // 2 - 1:
                    mid_cb()
            return dv

        HB = (NBLK // 2) * 128
        HQ = HB // 2

        def coll_pair(l):
            def fire_a():
                nc.gpsimd.collective_compute(
                    "AllGather", OP.bypass, ins=[cins[l][0:HB]],
                    outs=[couts[l][0:NCORE * HQ]], replica_groups=[list(range(NCORE))])

            def fire_b():
                nc.gpsimd.collective_compute(
                    "AllGather", OP.bypass, ins=[cins[l][HB:NPAD]],
                    outs=[couts[l][NCORE * HQ:TROWS // 2]], replica_groups=[list(range(NCORE))])

            return fire_a, fire_b

        # ================== forward ==================
        stack0 = stkp.tile([128, NPAD], F16, tag="stk")
        xsep = xsp.tile([64, NPAD], F16, tag="xsep")
        nc.sync.dma_start(stack0[0:64, 0:NPAD // 2], xT_in[:, 0:NPAD // 2])
        nc.scalar.dma_start(stack0[0:64, NPAD // 2:], xT_in[:, NPAD // 2:])
        fm_mm(xsep[:], lin1T_s[:], stack0[0:64, :], NPAD, func=AF.Lrelu, bias=lin1b_s[:])
        nc.sync.dma_start(stack0[64:128, :], xsep[:])
        rv = big.tile([128, NBLK], F32, tag="rvals0")
        ca, cb = coll_pair(0)
        build_rows(cins[0], 128, xsep[:], None, w1aT_s[:], attr_s[:], None, 0, dv=rv,
                   mid_cb=ca)
        cb()
        edge_phase(0, couts[0], rv, stack0)
        elu_inplace(stack0, gateb_s[:], NPAD, pre_lhsT=g2T_s[:])
        xcur, xsep_cur = stack0, xsep
        for l in range(2):
            xnew = stkp.tile([128, NPAD], F16, tag="stk")
            xsep_n = xsp.tile([64, NPAD], F16, tag="xsep")
            gru(xcur, xsep_cur, l, xsep_n[:], NPAD)
            nc.sync.dma_start(xnew[64:128, :], xsep_n[:])
            dv = big.tile([128, NBLK], F32, tag=f"rvals{l + 1}")
            ca, cb = coll_pair(l + 1)
            build_rows(cins[l + 1], 64, xsep_n[:], None, None,
                       adst_s[:, l], None, 0, dv=dv, mid_cb=ca)
            cb()
            edge_phase(l + 1, couts[l + 1], dv, xnew)
            elu_inplace(xnew, atomb_s[:, l:l + 1], NPAD, pre_lhsT=atomT_s[:, l])
            xcur, xsep_cur = xnew, xsep_n
        xfin = xsp.tile([64, NPAD], F16, tag="xsep")
        gru(xcur, xsep_cur, 2, xfin[:], NPAD)
        build_rows(routT, 128, xfin[:], None, None, None, None, 0)
        # readout: gather x slots from LOCAL table (resident rrow), on-chip
        # a_src (sres), masked out0 accumulation
        rrow = big.tile([128, GCH, 64], F16, tag="rrow")
        sres = big.tile([128, GCH, 1], F32, tag="sres")
        for b in range(GBLK):
            for (r0, r1) in gcalls[b]:
                cr = r1 - r0
                ch0 = gbases[b] + r0
                g = grp_.tile([128, MAXRG, 128], F16, tag="gr")
                nc.gpsimd.dma_gather(
                    g[:, 0:cr], routT[:], rgidx_s[:, ch0 * 8:(ch0 + cr) * 8],
                    cr * 128, cr * 128, 128, elem_step=128,
                    single_packet=False, queue_num=qctr[0] % 4)
                qctr[0] += 1
                nc.vector.tensor_mul(rrow[:, ch0:ch0 + cr], g[:, 0:cr, 0:64],
                                      rmsk16_s[:, ch0:ch0 + cr].to_broadcast([128, cr, 64]))
                sv = sp.tile([128, MAXRG, 64], F16, tag="svr")
                nc.vector.tensor_mul(sv[:, 0:cr], rrow[:, ch0:ch0 + cr],
                                     msrc_s[:].to_broadcast([128, cr, 64]))
                nc.vector.tensor_reduce(sres[:, ch0:ch0 + cr], sv[:, 0:cr], mybir.AxisListType.X, OP.add)
        ofm = big.tile([64, GPAD], F16, tag="ofm")
        hro = big.tile([64, GPAD], F16, tag="hro")
        mol_stk = big.tile([128, GPAD], F16, tag="mstk")
        for b in range(GBLK):
            ps0 = rp.tile([128, 64], F32, tag="red")
            RGb = int(RG[b])
            for r in range(RGb):
                nc.tensor.matmul(ps0[:], id16[:], rrow[:, gbases[b] + r],
                                 start=(r == 0), stop=(r == RGb - 1))
            s0 = sp.tile([128, 64], F32, tag="hnm")
            nc.scalar.activation(s0[:], ps0[:], AF.Copy)
            t0 = transp(s0[:], F32, "th")
            nc.scalar.activation(ofm[:, b * 128:(b + 1) * 128], t0[0:64, 0:128], AF.Relu)
        wtil_ps = rp.tile([64, 1], F32, tag="red")
        nc.tensor.matmul(wtil_ps[:], mol_lin_s[:], mdst_s[:], start=True, stop=True)
        wtil = cpool.tile([64, 1], F16)
        nc.vector.tensor_copy(wtil[:], wtil_ps[:])
        for t in range(3):
            ddp = rp.tile([1, GPAD], F32, tag="red")
            nc.tensor.matmul(ddp[:], wtil[:], ofm[:], start=True, stop=True)
            dds = s1.tile([1, GPAD], F32, tag="dds")
            nc.vector.tensor_copy(dds[:], ddp[:])
            for b in range(GBLK):
                ddb = transp(dds[:, b * 128:(b + 1) * 128], F32, "ddb")
                RGb = int(RG[b])
                gb0 = gbases[b]
                pred = rp.tile([128, 64], F32, tag="red")
                ddb001 = sp.tile([128, 1], F32, tag="ddb001")
                nc.vector.tensor_scalar(ddb001[:], ddb[:, 0:1], NS, None, OP.mult)
                e1 = sp.tile([128, GCH, 1], F32, tag="lgro")
                nc.scalar.activation(e1[:, gb0:gb0 + RGb], sres[:, gb0:gb0 + RGb],
                                     AF.Exp, bias=ddb[:, 0:1])
                e2 = sp.tile([128, GCH, 1], F32, tag="lgro2")
                nc.scalar.activation(e2[:, gb0:gb0 + RGb], sres[:, gb0:gb0 + RGb],
                                     AF.Exp, scale=NS, bias=ddb001[:, 0:1])
                nc.vector.tensor_max(e1[:, gb0:gb0 + RGb], e1[:, gb0:gb0 + RGb], e2[:, gb0:gb0 + RGb])
                p16 = sp.tile([128, GCH, 1], F16, tag="p16r")
                dsum = sp.tile([128, 1], F32, tag="dsum")
                nc.vector.scalar_tensor_tensor(
                    p16[:, gb0:gb0 + RGb], e1[:, gb0:gb0 + RGb], 1.0,
                    rmsk_s[:, gb0:gb0 + RGb], OP.mult, OP.mult, accum_out=dsum[:])
                msg = sp.tile([128, GCH, 64], F16, tag="msgr")
                nc.vector.tensor_mul(msg[:, gb0:gb0 + RGb], rrow[:, gb0:gb0 + RGb, 0:64],
                                     p16[:, gb0:gb0 + RGb].to_broadcast([128, RGb, 64]))
                for r in range(RGb):
                    nc.tensor.matmul(pred[:, 0:64], id16[:], msg[:, gb0 + r],
                                     start=(r == 0), stop=(r == RGb - 1))
                rec = sp.tile([128, 1], F32, tag="rec")
                nc.vector.tensor_scalar(rec[:], dsum[:], 1e-16, None, OP.add)
                nc.vector.reciprocal(rec[:], rec[:])
                hnm = sp.tile([128, 64], F32, tag="hnm")
                nc.scalar.activation(hnm[:], pred[:, 0:64], AF.Copy, scale=rec[:])
                th = transp(hnm[:], F32, "th")
                nc.scalar.activation(hro[:, b * 128:(b + 1) * 128], th[0:64, 0:128], AF.Copy)
            elu_inplace(hro, molb_s[:], GPAD, pre_lhsT=molT_s[:])
            nc.vector.tensor_copy(mol_stk[0:64, :], hro[:])
            nc.vector.tensor_copy(mol_stk[64:128, :], ofm[:])
            onew = s1.tile([64, GPAD], F16, tag="onew")
            gru(mol_stk, ofm, 3, onew[:], GPAD)
            nc.vector.tensor_copy(ofm[:], onew[:])
        emb = sp.tile([64, GPAD], F32, tag="emb")
        fm_mm(emb[:], lin2T_s[:], ofm[:], GPAD, bias=lin2b_s[:])
        nemb = sp.tile([64, GPAD], F16, tag="nemb")
        for b in range(GBLK):
            gm = transp(emb[:, b * 128:(b + 1) * 128], F32, "gm")
            mu = sp.tile([128, 1], F32, tag="mu")
            nc.vector.tensor_reduce(mu[:], gm[:, 0:64], mybir.AxisListType.X, OP.add)
            nc.vector.tensor_scalar(mu[:], mu[:], 1.0 / 64, None, OP.mult)
            xc = sp.tile([128, 64], F32, tag="xc")
            nc.vector.tensor_scalar(xc[:], gm[:, 0:64], mu[:], None, OP.subtract)
            sq = sp.tile([128, 64], F32, tag="sq")
            nc.scalar.activation(sq[:], xc[:], AF.Square)
            var = sp.tile([128, 1], F32, tag="var")
            nc.vector.tensor_reduce(var[:], sq[:], mybir.AxisListType.X, OP.add)
            nc.vector.tensor_scalar(var[:], var[:], 1.0 / 64, None, OP.mult)
            nc.vector.tensor_scalar(var[:], var[:], 1e-5, None, OP.add)
            nc.scalar.activation(var[:], var[:], AF.Sqrt)
            nc.vector.reciprocal(var[:], var[:])
            nc.scalar.activation(xc[:], xc[:], AF.Copy, scale=var[:])
            nc.vector.tensor_mul(xc[:], xc[:], lng_s[:, 0:64])
            nc.vector.tensor_add(xc[:], xc[:], lnb_s[:, 0:64])
            tb = transp(xc[:], F16, "tb2")
            nc.vector.tensor_copy(nemb[:, b * 128:(b + 1) * 128], tb[0:64, 0:128])
        m1 = sp.tile([64, GPAD], F16, tag="m1")
        fm_mm(m1[:], h1T_s[:], nemb[:], GPAD, func=AF.Relu, bias=h1b_s[:])
        m2 = sp.tile([64, GPAD], F16, tag="m2")
        fm_mm(m2[:], h2T_s[:], m1[:], GPAD, func=AF.Relu, bias=h2b_s[:])
        yps = rp.tile([1, GPAD], F32, tag="red")
        nc.tensor.matmul(yps[:], h3T_s[:], m2[:], start=True, stop=True)
        ysb = s1.tile([1, GPAD], F32, tag="ysb")
        nc.scalar.activation(ysb[:], yps[:], AF.Identity, bias=h3b_s[:])
        nc.sync.dma_start(y_out[:], ysb[:])
        ctx.close()
    ctx2.__exit__(None, None, None)
    nc.finalize()
    return nc


_CACHE = {}


def kernel(**inputs):
    x = np.asarray(inputs["x"], np.float32)
    edge_attr = np.asarray(inputs["edge_attr"], np.float32)
    ei = np.asarray(inputs["edge_index"])
    batch = np.asarray(inputs["batch"])
    if "k" not in _CACHE:
        plan = build_plan(ei, batch)
        nc = build_nc(plan)
        _CACHE["k"] = (plan, nc)
    plan, nc = _CACHE["k"]
    NPAD = plan["NPAD"]

    gw = np.zeros((128, 4, 128), np.float16)
    gn = np.zeros((128, 4, 128), np.float16)
    gbx_rz = np.zeros((64, 8), np.float32)
    gbh_rz = np.zeros((64, 8), np.float32)
    gbx_n = np.zeros((H, 4), np.float32)
    gbh_n = np.zeros((H, 4), np.float32)
    packs = [
        (inputs["gru0_wx"], inputs["gru0_wh"], inputs["gru0_bx"], inputs["gru0_bh"]),
        (inputs["atom_gru_wx"][0], inputs["atom_gru_wh"][0], inputs["atom_gru_bx"][0], inputs["atom_gru_bh"][0]),
        (inputs["atom_gru_wx"][1], inputs["atom_gru_wh"][1], inputs["atom_gru_bx"][1], inputs["atom_gru_bh"][1]),
        (inputs["mol_gru_wx"], inputs["mol_gru_wh"], inputs["mol_gru_bx"], inputs["mol_gru_bh"]),
    ]
    for i, (wx, wh, bx, bh) in enumerate(packs):
        wx = np.asarray(wx, np.float32); wh = np.asarray(wh, np.float32)
        bx = np.asarray(bx, np.float32); bh = np.asarray(bh, np.float32)
        gw[0:64, i, 0:64] = wx[0:64].T; gw[64:128, i, 0:64] = wh[0:64].T
        gw[0:64, i, 64:128] = wx[64:128].T; gw[64:128, i, 64:128] = wh[64:128].T
        gn[0:64, i, 0:64] = wx[128:192].T; gn[64:128, i, 64:128] = wh[128:192].T
        gbx_rz[:, 2 * i] = bx[0:64]; gbx_rz[:, 2 * i + 1] = bx[64:128]
        gbh_rz[:, 2 * i] = bh[0:64]; gbh_rz[:, 2 * i + 1] = bh[64:128]
        gbx_n[:, i] = bx[128:192]; gbh_n[:, i] = bh[128:192]

    glw = np.asarray(inputs["gate_lin1_w"], np.float32)
    rep16 = lambda v: np.tile(np.asarray(v, np.float16).reshape(1, -1), (128, 1))
    rep32 = lambda v: np.tile(np.asarray(v, np.float32).reshape(1, -1), (128, 1))
    a = lambda k: np.asarray(inputs[k], np.float32)
    f16 = lambda v: np.asarray(v, np.float16)
    wts = dict(
        lin1T=f16(a("lin1_w").T), lin1_b=a("lin1_b").reshape(H, 1),
        w1aT=f16(glw[:, 0:64].T),
        w1bT=f16(glw[:, 64:80].T),
        attl_rep=rep16(inputs["gate_att_l"]).reshape(128, 1, H),
        attr_rep=rep16(inputs["gate_att_r"]),
        g2T=f16(a("gate_lin2_w").T), gate_b=a("gate_bias").reshape(H, 1),
        atomT=f16(np.stack([a("atom_lin_w")[l].T for l in range(2)], 1)),
        asrc_rep=np.stack([rep16(a("atom_lin_w")[l].T @ a("atom_att_src")[l]) for l in range(2)], 1),
        adst_rep=np.stack([rep16(a("atom_lin_w")[l].T @ a("atom_att_dst")[l]) for l in range(2)], 1),
        atom_b=a("atom_bias").T.copy(),
        molT=f16(a("mol_lin_w").T), mol_lin=a("mol_lin_w").copy(),
        matt_src_rep=rep16(a("mol_lin_w").T @ a("mol_att_src")).reshape(128, 1, H),
        matt_dst=a("mol_att_dst").reshape(H, 1),
        mol_b=a("mol_bias").reshape(H, 1),
        gruW=gw, gruN=gn, gbx_rz=gbx_rz, gbh_rz=gbh_rz, gbx_n=gbx_n, gbh_n=gbh_n,
        lin2T=f16(a("lin2_w").T), lin2_b=a("lin2_b").reshape(H, 1),
        lng_rep=rep32(inputs["ln_g"]), lnb_rep=rep32(inputs["ln_b"]),
        h1T=f16(a("h1_w").T), h1_b=a("h1_b").reshape(H, 1),
        h2T=f16(a("h2_w").T), h2_b=a("h2_b").reshape(H, 1),
        h3T=f16(a("h3_w").T), h3_b=a("h3_b").reshape(1, 1),
    )
    in_maps = []
    for c in range(NCORE):
        pc = plan["cores"][c]
        rc = plan["rcores"][c]
        lo, hi = int(plan["bounds"][c]), int(plan["bounds"][c + 1])
        xT = np.zeros((H, NPAD), np.float16)
        xT[:, 0:hi - lo] = x[lo:hi][plan["perms"][c]].T
        eaT = np.zeros((16, plan["S"]), np.float16)
        val = pc["esel"] >= 0
        eaT[:, val] = edge_attr[pc["esel"][val]].T.astype(np.float16)
        im = dict(xT_in=xT, gidx=pc["gidx"], par=pc["par"], msk=pc["msk"], eaT=eaT,
                  rgidx=rc["rgidx"], rmsk=rc["rmsk"], rmsk16=rc["rmsk16"], **wts)
        in_maps.append(im)
    res = run_bass_kernel_spmd(nc, in_maps, core_ids=list(range(NCORE)))
    if getattr(res, "exec_time_ns", None):
        print(f"HW exec time: {res.exec_time_ns} ns", flush=True)
    y = np.zeros(B, np.float32)
    for c in range(NCORE):
        ys = res.results[c]["y_out"].reshape(GPAD)[0:GPC]
        y[c * GPC + plan["gperms"][c]] = ys
    return y.reshape(B, 1)


# revision 46
# speedup vs baseline: 1.0156x; 1.0156x over previous
"""AttentiveFP forward on 8 Trainium2 NeuronCores (Bass/Tile).

Nodes sharded at graph boundaries (batch is sorted), so every graph's nodes
live on one core and the attentive readout is fully local (no readout
collective, unpaired local gathers). Edges sharded by dst-owner core;
per-core nodes sorted by in-degree with a round-robin slot structure so
segment softmax/sum become dense PSUM matmul accumulation.

Per layer: SBUF-resident gather index table (loaded once, shared by all
layers), paired-256/512B dma_gather of fp16 rows (idx = table_row>>1 fits
int16; parity select on DVE), softmax logit chain via the identity
exp(leaky(x)) = max(exp(x), exp(0.01 x)) so the ACT engine never switches
function tables, denominators fused into the alpha mask multiply via
scalar_tensor_tensor accum_out, identity-matmul numerator reduction in
PSUM, fp16 GRU (2 fused 128-row gate matmuls), and a chunk-major two-piece
AllGather of the fp16 row table fired mid-build so transfers overlap the
row construction. Readout gathers from a local table; per-block batched
attention chains over all timestep slots.
"""
import numpy as np
from contextlib import ExitStack

import concourse.bass as bass
import concourse.tile as tile
from concourse import bacc, mybir
from concourse.bass_utils import run_bass_kernel_spmd
from concourse.masks import make_identity

F32 = mybir.dt.float32
F16 = mybir.dt.float16
I16 = mybir.dt.int16
AF = mybir.ActivationFunctionType
OP = mybir.AluOpType

NCORE = 8
N, E, B = 50000, 500000, 2048
H = 64
NS = 0.01
GPC = B // NCORE
GBLK = (GPC + 127) // 128
GPAD = GBLK * 128
MAXR0 = 10
MAXR2 = 20
MAXRG = 8
CH = 512


def _calls_for(R, maxr):
    calls, base, bases = [], 0, []
    for r in R:
        bases.append(base)
        calls.append([(r0, min(r0 + maxr, int(r))) for r0 in range(0, int(r), maxr)])
        base += int(r)
    return calls, bases, base


def _wrap_into(gidx, arr, col0):
    n = arr.shape[0]
    blk = arr.reshape(n // 16, 16).T
    gidx[:16, col0:col0 + n // 16] = blk
    gidx[16:128, col0:col0 + n // 16] = np.tile(blk, (7, 1))


def build_plan(edge_index, batch):
    src = edge_index[0].astype(np.int64)
    dst = edge_index[1].astype(np.int64)

    gsize = np.bincount(batch, minlength=B)
    gstart = np.concatenate([[0], np.cumsum(gsize)])
    bounds = np.array([gstart[c * GPC] for c in range(NCORE)] + [N], np.int64)
    npcs = [int(bounds[c + 1] - bounds[c]) for c in range(NCORE)]
    NBLK = (max(npcs) + 127) // 128
    NPAD = NBLK * 128
    TROWS = NCORE * NPAD

    owner = np.searchsorted(bounds[1:], dst, side="right")

    perms, degs_sorted, grp_starts, egrp = [], [], [], []
    sortpos = np.zeros(N, np.int64)
    for c in range(NCORE):
        n0 = bounds[c]
        emask = np.nonzero(owner == c)[0]
        deg = np.bincount(dst[emask] - n0, minlength=npcs[c])
        order = np.argsort(-deg, kind="stable")
        perms.append(order)
        sortpos[n0 + order] = np.arange(npcs[c])
        dsorted = deg[order]
        degs_sorted.append(dsorted)
        eorder = np.argsort(sortpos[dst[emask]], kind="stable")
        egrp.append(emask[eorder])
        grp_starts.append(np.concatenate([[0], np.cumsum(dsorted)]))
    node_owner = np.searchsorted(bounds[1:], np.arange(N), side="right")
    HQ = (NBLK // 2) * 64            # pair-rows per core in collective chunk A
    q = sortpos >> 1
    prow = np.where(q < HQ, node_owner * HQ + q,
                    NCORE * HQ + node_owner * (NPAD // 2 - HQ) + (q - HQ))
    parbit = sortpos & 1

    R = np.ones(NBLK, np.int64)
    for b in range(NBLK):
        for c in range(NCORE):
            d = degs_sorted[c][b * 128:(b + 1) * 128]
            if len(d):
                R[b] = max(R[b], int(d[0]))
    calls, bases, NCH = _calls_for(R, MAXR0)
    calls2, _, _ = _calls_for(R, MAXR2)
    S = NCH * 128

    gperms, gss = [], []
    for c in range(NCORE):
        gs = gsize[c * GPC:(c + 1) * GPC]
        gorder = np.argsort(-gs, kind="stable")
        gperms.append(gorder)
        gss.append(gs[gorder])
    RG = np.ones(GBLK, np.int64)
    for b in range(GBLK):
        for c in range(NCORE):
            d = gss[c][b * 128:(b + 1) * 128]
            if len(d):
                RG[b] = max(RG[b], int(d[0]))
    gcalls, gbases, GCH = _calls_for(RG, MAXRG)
    SR = GCH * 128

    cores = []
    lanes = np.arange(128)
    for c in range(NCORE):
        gidx = np.zeros((128, S // 16), np.int16)
        par = np.zeros((128, NCH, 1), np.float16)
        msk = np.zeros((128, NCH, 1), np.float16)
        esel = np.full(NCH * 128, -1, np.int64)
        ds = degs_sorted[c]
        gst = grp_starts[c]
        eg = egrp[c]
        npc = npcs[c]
        for b in range(NBLK):
            for (r0, r1) in calls[b]:
                ia = np.zeros((r1 - r0) * 128, np.int64)
                for r in range(r0, r1):
                    ch = bases[b] + r
                    p = b * 128 + lanes
                    pc = np.minimum(p, npc - 1)
                    ok = (p < npc) & (r < ds[pc])
                    eids = gst[pc] + r
                    ge = np.where(ok, eg[np.where(ok, np.minimum(eids, len(eg) - 1), 0)], -1)
                    esel[ch * 128 + lanes] = ge
                    sj = src[np.maximum(ge, 0)]
                    ia[(r - r0) * 128 + lanes] = np.where(ok, prow[sj], 0)
                    par[:, ch, 0] = np.where(ok, parbit[sj], 0).astype(np.float16)
                    msk[:, ch, 0] = ok.astype(np.float32)
                _wrap_into(gidx, ia, (bases[b] + r0) * 8)
        cores.append(dict(gidx=gidx, par=par, msk=msk, esel=esel))

    # readout plan: fully local (graph-aligned sharding), unpaired row idx
    rcores = []
    for c in range(NCORE):
        rgidx = np.zeros((128, SR // 16), np.int16)
        rmsk = np.zeros((128, GCH, 1), np.float32)
        rmsk16 = np.zeros((128, GCH, 1), np.float16)
        gs = gss[c]
        gp = gperms[c]
        for b in range(GBLK):
            for (r0, r1) in gcalls[b]:
                ia = np.zeros((r1 - r0) * 128, np.int64)
                for r in range(r0, r1):
                    ch = gbases[b] + r
                    p = b * 128 + lanes
                    pc = np.minimum(p, GPC - 1)
                    ok = (p < GPC) & (r < gs[pc])
                    g = c * GPC + gp[pc]
                    node = np.where(ok, gstart[g] + r, bounds[c])
                    ia[(r - r0) * 128 + lanes] = sortpos[node]
                    rmsk[:, ch, 0] = ok.astype(np.float32)
                    rmsk16[:, ch, 0] = ok.astype(np.float16)
                _wrap_into(rgidx, ia, (gbases[b] + r0) * 8)
        rcores.append(dict(rgidx=rgidx, rmsk=rmsk, rmsk16=rmsk16))

    return dict(R=R, calls=calls, calls2=calls2, bases=bases, NCH=NCH, S=S,
                RG=RG, gcalls=gcalls, gbases=gbases, GCH=GCH, SR=SR,
                NBLK=NBLK, NPAD=NPAD, TROWS=TROWS, bounds=bounds, npcs=npcs,
                cores=cores, rcores=rcores, perms=perms, gperms=gperms)


def build_nc(plan):
    R, calls, bases, NCH, S = plan["R"], plan["calls"], plan["bases"], plan["NCH"], plan["S"]
    calls2 = plan["calls2"]
    RG, gcalls, gbases, GCH, SR = plan["RG"], plan["gcalls"], plan["gbases"], plan["GCH"], plan["SR"]
    NBLK, NPAD, TROWS = plan["NBLK"], plan["NPAD"], plan["TROWS"]

    nc = bacc.Bacc("TRN2", target_bir_lowering=False, debug=False,
                   num_devices=NCORE, num_swdge_queues=4)

    def din(name, shape, dt=F32):
        return nc.dram_tensor(name, shape, dt, kind="ExternalInput")

    xT_in = din("xT_in", [H, NPAD], F16)
    gidx_in = din("gidx", [128, S // 16], I16)
    par_in = din("par", [128, NCH, 1], F16)
    msk_in = din("msk", [128, NCH, 1], F16)
    eaT_in = din("eaT", [16, S], F16)
    rgidx_in = din("rgidx", [128, SR // 16], I16)
    rmsk_in = din("rmsk", [128, GCH, 1], F32)
    rmsk16_in = din("rmsk16", [128, GCH, 1], F16)
    lin1T = din("lin1T", [H, H], F16); lin1_b = din("lin1_b", [H, 1])
    w1aT = din("w1aT", [H, H], F16); w1bT = din("w1bT", [16, H], F16)
    attl_rep = din("attl_rep", [128, 1, H], F16)
    attr_rep = din("attr_rep", [128, H], F16)
    g2T = din("g2T", [H, H], F16); gate_b = din("gate_b", [H, 1])
    atomT = din("atomT", [H, 2, H], F16)
    asrc_rep = din("asrc_rep", [128, 2, H], F16)
    adst_rep = din("adst_rep", [128, 2, H], F16)
    atom_b = din("atom_b", [H, 2])
    molT = din("molT", [H, H], F16)
    mol_lin = din("mol_lin", [H, H])
    matt_src_rep = din("matt_src_rep", [128, 1, H], F16)
    matt_dst = din("matt_dst", [H, 1])
    mol_b = din("mol_b", [H, 1])
    gruW = din("gruW", [128, 4, 128], F16)   # [K, widx, r|z]
    gruN = din("gruN", [128, 4, 128], F16)   # [K, widx, nx|nh] zero-padded
    gbx_rz = din("gbx_rz", [64, 8])
    gbh_rz = din("gbh_rz", [64, 8])
    gbx_n = din("gbx_n", [H, 4])
    gbh_n = din("gbh_n", [H, 4])
    lin2T = din("lin2T", [H, H], F16); lin2_b = din("lin2_b", [H, 1])
    lng_rep = din("lng_rep", [128, H]); lnb_rep = din("lnb_rep", [128, H])
    h1T = din("h1T", [H, H], F16); h1_b = din("h1_b", [H, 1])
    h2T = din("h2T", [H, H], F16); h2_b = din("h2_b", [H, 1])
    h3T = din("h3T", [H, 1], F16); h3_b = din("h3_b", [1, 1])

    y_out = nc.dram_tensor("y_out", [1, GPAD], F32, kind="ExternalOutput")

    cins, couts = [], []
    for l in range(3):
        cw = 128 if l == 0 else 64
        cins.append(nc.dram_tensor(f"cin{l}", [NPAD, cw], F16))
        couts.append(nc.dram_tensor(f"cout{l}", [TROWS // 2, 2 * cw], F16,
                                    addr_space="Shared"))
    routT = nc.dram_tensor("routT", [NPAD, 128], F16)

    ctx = ExitStack()
    ctx2 = nc.allow_low_precision(reason="fp16 edge tables/messages")
    ctx2.__enter__()
    with tile.TileContext(nc) as tc:
        cpool = ctx.enter_context(tc.tile_pool(name="const", bufs=1))
        wpool = ctx.enter_context(tc.tile_pool(name="wts", bufs=1))
        big = ctx.enter_context(tc.tile_pool(name="big", bufs=1))
        stkp = ctx.enter_context(tc.tile_pool(name="stkp", bufs=2))
        xsp = ctx.enter_context(tc.tile_pool(name="xsp", bufs=2))
        gp = ctx.enter_context(tc.tile_pool(name="gath", bufs=5))
        grp_ = ctx.enter_context(tc.tile_pool(name="gathr", bufs=3))
        sp = ctx.enter_context(tc.tile_pool(name="scr", bufs=2))
        sp3 = ctx.enter_context(tc.tile_pool(name="scr3", bufs=3))
        s1 = ctx.enter_context(tc.tile_pool(name="scr1", bufs=1))
        rowp = ctx.enter_context(tc.tile_pool(name="rows", bufs=4))
        pp = ctx.enter_context(tc.tile_pool(name="ps", bufs=2, space="PSUM"))
        rp = ctx.enter_context(tc.tile_pool(name="psr", bufs=2, space="PSUM"))
        zp = ctx.enter_context(tc.tile_pool(name="psz", bufs=1, space="PSUM"))

        id32 = cpool.tile([128, 128], F32)
        make_identity(nc, id32[:])
        id16 = cpool.tile([128, 128], F16)
        nc.vector.tensor_copy(id16[:], id32[:])

        _ldctr = [0]

        def load(t, shape, dt=F32):
            s = wpool.tile(shape, dt, tag=f"w_{t.name}")
            eng = nc.sync if _ldctr[0] % 2 == 0 else nc.scalar
            _ldctr[0] += 1
            eng.dma_start(s[:], t[:])
            return s

        gidx_s = load(gidx_in, [128, S // 16], I16)
        rgidx_s = load(rgidx_in, [128, SR // 16], I16)
        par_s = load(par_in, [128, NCH, 1], F16)
        msk_s = load(msk_in, [128, NCH, 1], F16)
        rmsk_s = load(rmsk_in, [128, GCH, 1], F32)
        rmsk16_s = load(rmsk16_in, [128, GCH, 1], F16)
        lin1T_s = load(lin1T, [H, H], F16); lin1b_s = load(lin1_b, [H, 1])
        w1aT_s = load(w1aT, [H, H], F16); w1bT_s = load(w1bT, [16, H], F16)
        attl_s = load(attl_rep, [128, 1, H], F16)
        attr_s = load(attr_rep, [128, H], F16)
        g2T_s = load(g2T, [H, H], F16); gateb_s = load(gate_b, [H, 1])
        atomT_s = load(atomT, [H, 2, H], F16)
        asrc_s = load(asrc_rep, [128, 2, H], F16)
        adst_s = load(adst_rep, [128, 2, H], F16)
        atomb_s = load(atom_b, [H, 2])
        molT_s = load(molT, [H, H], F16); mol_lin_s = load(mol_lin, [H, H])
        msrc_s = load(matt_src_rep, [128, 1, H], F16)
        mdst_s = load(matt_dst, [H, 1])
        molb_s = load(mol_b, [H, 1])
        gruW_s = load(gruW, [128, 4, 128], F16)
        gruN_s = load(gruN, [128, 4, 128], F16)
        gbxrz_s = load(gbx_rz, [64, 8])
        gbhrz_s = load(gbh_rz, [64, 8])
        gbxn_s = load(gbx_n, [H, 4])
        gbhn_s = load(gbh_n, [H, 4])
        lin2T_s = load(lin2T, [H, H], F16); lin2b_s = load(lin2_b, [H, 1])
        lng_s = load(lng_rep, [128, H]); lnb_s = load(lnb_rep, [128, H])
        h1T_s = load(h1T, [H, H], F16); h1b_s = load(h1_b, [H, 1])
        h2T_s = load(h2T, [H, H], F16); h2b_s = load(h2_b, [H, 1])
        h3T_s = load(h3T, [H, 1], F16); h3b_s = load(h3_b, [1, 1])

        def fm_mm(out_ap, lhsT, rhs, ncols, func=None, bias=0.0):
            M = lhsT.shape[-1]
            for c0 in range(0, ncols, CH):
                w = min(CH, ncols - c0)
                ps = pp.tile([128, CH], F32, tag="mmq")
                nc.tensor.matmul(ps[0:M, :w], lhsT, rhs[:, c0:c0 + w],
                                 start=True, stop=True)
                f = func
                if f is None:
                    f = AF.Copy if isinstance(bias, float) else AF.Identity
                nc.scalar.activation(out_ap[:, c0:c0 + w], ps[0:M, :w], f, bias=bias)

        def transp(in_ap, out_dt, tag):
            a, bdim = in_ap.shape[0], in_ap.shape[-1]
            ps = rp.tile([128, 128], F32, tag="tp")
            if in_ap.dtype == F16:
                pv = ps[:].bitcast(F16)
                nc.tensor.transpose(pv[0:bdim, 0:a], in_ap, id16[0:a, 0:a])
                src = pv[0:bdim, 0:a]
            else:
                nc.tensor.transpose(ps[0:bdim, 0:a], in_ap, id32[0:a, 0:a])
                src = ps[0:bdim, 0:a]
            t = sp.tile([128, 128], out_dt, tag=tag)
            nc.scalar.activation(t[0:bdim, 0:a], src, AF.Copy)
            return t

        def gru(stacked, xsep, widx, out_ap, ncols):
            br = sp.tile([64, 1], F32, tag="brz")
            nc.vector.tensor_add(br[:], gbxrz_s[:, 2 * widx:2 * widx + 1], gbhrz_s[:, 2 * widx:2 * widx + 1])
            bz = sp.tile([64, 1], F32, tag="bz")
            nc.vector.tensor_add(bz[:], gbxrz_s[:, 2 * widx + 1:2 * widx + 2], gbhrz_s[:, 2 * widx + 1:2 * widx + 2])
            for c0 in range(0, ncols, CH):
                w = min(CH, ncols - c0)
                prz = pp.tile([128, CH], F32, tag="mmq")
                nc.tensor.matmul(prz[:, :w], gruW_s[:, widx], stacked[:, c0:c0 + w], start=True, stop=True)
                rz = sp3.tile([64, CH], F16, tag="rz")
                nc.scalar.activation(rz[:, :w], prz[0:64, :w], AF.Sigmoid, bias=br[:])
                zz = sp3.tile([64, CH], F16, tag="zz")
                nc.scalar.activation(zz[:, :w], prz[64:128, :w], AF.Sigmoid, bias=bz[:])
                pn = pp.tile([128, CH], F32, tag="mmq")
                nc.tensor.matmul(pn[:, :w], gruN_s[:, widx], stacked[:, c0:c0 + w], start=True, stop=True)
                hnb = sp3.tile([64, CH], F16, tag="hnb")
                nc.scalar.activation(hnb[:, :w], pn[64:128, :w], AF.Identity, bias=gbhn_s[:, widx:widx + 1])
                t1 = sp3.tile([64, CH], F32, tag="t1")
                nc.vector.tensor_mul(t1[:, :w], rz[:, :w], hnb[:, :w])
                nc.vector.tensor_add(t1[:, :w], t1[:, :w], pn[0:64, :w])
                nn = sp3.tile([64, CH], F16, tag="nn")
                nc.scalar.activation(nn[:, :w], t1[:, :w], AF.Tanh, bias=gbxn_s[:, widx:widx + 1])
                d = sp3.tile([64, CH], F16, tag="dd1")
                nc.vector.tensor_sub(d[:, :w], xsep[:, c0:c0 + w], nn[:, :w])
                nc.vector.tensor_mul(d[:, :w], zz[:, :w], d[:, :w])
                nc.vector.tensor_add(d[:, :w], nn[:, :w], d[:, :w])
                nc.scalar.activation(out_ap[:, c0:c0 + w], d[:, :w], AF.Relu)

        def elu_inplace(buf, bias_ap, ncols, pre_lhsT=None):
            """buf[0:64, :] = elu((pre_lhsT.T @ buf[0:64]) + bias)."""
            for c0 in range(0, ncols, CH):
                w = min(CH, ncols - c0)
                if pre_lhsT is not None:
                    ps = pp.tile([128, CH], F32, tag="mmq")
                    nc.tensor.matmul(ps[0:64, :w], pre_lhsT, buf[0:64, c0:c0 + w], start=True, stop=True)
                    src = ps[0:64, :w]
                else:
                    src = buf[0:64, c0:c0 + w]
                e1 = sp.tile([64, CH], F16, tag="e1")
                nc.scalar.activation(e1[:, :w], src, AF.Exp, bias=bias_ap)
                t1 = sp.tile([64, CH], F16, tag="el1")
                nc.scalar.activation(t1[:, :w], e1[:, :w], AF.Relu, bias=1.0, scale=-1.0)
                t2 = sp.tile([64, CH], F16, tag="el2")
                nc.scalar.activation(t2[:, :w], src, AF.Relu, bias=bias_ap)
                nc.vector.tensor_sub(buf[0:64, c0:c0 + w], t2[:, :w], t1[:, :w])

        qctr = [0]

        def edge_phase(layer, table, dvals, hdst):
            """hdst[0:64, :NPAD] <- normalized aggregation (feature-major)."""
            MX = MAXR0 if layer == 0 else MAXR2
            EW = 256 if layer == 0 else 128
            W = 128 if layer == 0 else 64
            lcalls = calls if layer == 0 else calls2
            dv001 = sp.tile([128, NBLK], F32, tag="dv001")
            nc.vector.tensor_scalar(dv001[:], dvals[:], NS, None, OP.mult)
            for b in range(NBLK):
                pred = rp.tile([128, 64], F32, tag="red")
                dsum = sp.tile([128, 1], F32, tag="dsum")
                nc.vector.memset(dsum[:], 1e-16)
                Rb = int(R[b])
                for (r0, r1) in lcalls[b]:
                    cr = r1 - r0
                    ch0 = bases[b] + r0
                    graw = gp.tile([128, 2560], F16, tag="g")
                    g = graw[:].rearrange("p (a b) -> p a b", b=EW)
                    nc.gpsimd.dma_gather(
                        g[:, 0:cr], table[:], gidx_s[:, ch0 * 8:(ch0 + cr) * 8],
                        cr * 128, cr * 128, EW, elem_step=EW,
                        single_packet=False, queue_num=qctr[0] % 4)
                    qctr[0] += 1
                    row = sp.tile([128, MX, W], F16, tag="row")
                    nc.vector.tensor_sub(row[:, 0:cr], g[:, 0:cr, W:2 * W], g[:, 0:cr, 0:W])
                    nc.vector.tensor_mul(row[:, 0:cr], row[:, 0:cr],
                                         par_s[:, ch0:ch0 + cr].to_broadcast([128, cr, W]))
                    nc.vector.tensor_add(row[:, 0:cr], row[:, 0:cr], g[:, 0:cr, 0:W])
                    lg = sp.tile([128, MX, 1], F32, tag="lg")
                    if layer == 0:
                        ea = s1.tile([16, MAXR0 * 128], F16, tag="ea")
                        eng = nc.sync if b % 2 == 0 else nc.scalar
                        eng.dma_start(ea[:, 0:cr * 128], eaT_in[:, ch0 * 128:(ch0 + cr) * 128])
                        pz = zp.tile([128, 10, 64], F32, tag="z1")
                        for r in range(cr):
                            nc.tensor.matmul(pz[:, r], ea[:, r * 128:(r + 1) * 128],
                                             w1bT_s[:], start=True, stop=True)
                        ev = sp.tile([128, 10, 64], F16, tag="ev")
                        nc.vector.tensor_add(ev[:, 0:cr], row[:, 0:cr, 64:128], pz[:, 0:cr])
                        evn = sp.tile([128, 10, 64], F16, tag="lr")
                        nc.scalar.activation(evn[:, 0:cr], ev[:, 0:cr], AF.Relu, scale=-(1.0 - NS))
                        nc.vector.tensor_add(ev[:, 0:cr], ev[:, 0:cr], evn[:, 0:cr])
                        sv = sp.tile([128, 10, 64], F16, tag="sv0")
                        nc.vector.tensor_mul(sv[:, 0:cr], ev[:, 0:cr],
                                             attl_s[:].to_broadcast([128, cr, 64]))
                        nc.vector.tensor_reduce(lg[:, 0:cr], sv[:, 0:cr], mybir.AxisListType.X, OP.add)
                        lg_ap = lg[:, 0:cr]
                    else:
                        sv = sp.tile([128, MX, 64], F16, tag="sv")
                        nc.vector.tensor_mul(sv[:, 0:cr], row[:, 0:cr, 0:64],
                                             asrc_s[:, layer - 1:layer].to_broadcast([128, cr, 64]))
                        nc.vector.tensor_reduce(lg[:, 0:cr], sv[:, 0:cr], mybir.AxisListType.X, OP.add)
                        lg_ap = lg[:, 0:cr]
                    e1 = sp.tile([128, MX, 1], F32, tag="lgl")
                    nc.scalar.activation(e1[:, 0:cr], lg_ap, AF.Exp, bias=dvals[:, b:b + 1])
                    e2 = sp.tile([128, MX, 1], F32, tag="lgl2")
                    nc.scalar.activation(e2[:, 0:cr], lg_ap, AF.Exp, scale=NS, bias=dv001[:, b:b + 1])
                    nc.vector.tensor_max(e1[:, 0:cr], e1[:, 0:cr], e2[:, 0:cr])
                    p16 = sp.tile([128, MX, 1], F16, tag="p16")
                    dcall = sp.tile([128, 1], F32, tag="dcall")
                    nc.vector.scalar_tensor_tensor(
                        p16[:, 0:cr], e1[:, 0:cr], 1.0, msk_s[:, ch0:ch0 + cr],
                        OP.mult, OP.mult, accum_out=dcall[:])
                    nc.vector.tensor_add(dsum[:], dsum[:], dcall[:])
                    msg = sp.tile([128, MX, 64], F16, tag="msg")
                    nc.vector.tensor_mul(msg[:, 0:cr], row[:, 0:cr, 0:64],
                                         p16[:, 0:cr].to_broadcast([128, cr, 64]))
                    for r in range(cr):
                        nc.tensor.matmul(pred[:, 0:64], id16[:], msg[:, r],
                                         start=(r0 + r == 0), stop=(r0 + r == Rb - 1))
                rec = sp.tile([128, 1], F32, tag="rec")
                nc.vector.reciprocal(rec[:], dsum[:])
                hnm = sp.tile([128, 64], F32, tag="hnm")
                nc.scalar.activation(hnm[:], pred[:, 0:64], AF.Copy, scale=rec[:])
                ps = rp.tile([128, 128], F32, tag="tp")
                nc.tensor.transpose(ps[0:64, 0:128], hnm[:], id32[:])
                nc.scalar.activation(hdst[0:64, b * 128:(b + 1) * 128], ps[0:64, 0:128], AF.Copy)

        def build_rows(cin_t, width, x_fm, lhsT_A, lhsT_B, dst_rep, src_rep, src_col,
                       dv=None, mid_cb=None):
            cview = cin_t[:].rearrange("(b p) e -> b p e", p=128)
            """rows[:, b, 0:64] = (A@x_b).T (A None -> x_b.T); cols 64:128 =
            (B@x_b).T (B "plain" -> x_b.T, None -> skip); aux dots appended."""
            for b in range(NBLK):
                rows_b = rowp.tile([128, 256], F16, tag="rb")
                if x_fm.base_partition() != 0:
                    xb0 = sp.tile([64, 128], F16, tag="xb0")
                    nc.vector.tensor_copy(xb0[:], x_fm[:, b * 128:(b + 1) * 128])
                    xb_ap = xb0[:]
                else:
                    xb_ap = x_fm[:, b * 128:(b + 1) * 128]
                if lhsT_A is not None:
                    ps = pp.tile([128, CH], F32, tag="mmq")
                    nc.tensor.matmul(ps[0:64, 0:128], lhsT_A, xb_ap, start=True, stop=True)
                    xa = sp.tile([64, 128], F16, tag="xb")
                    nc.scalar.activation(xa[:], ps[0:64, 0:128], AF.Copy)
                    src_fm = xa[:]
                else:
                    src_fm = xb_ap
                pst = rp.tile([128, 128], F32, tag="tp")
                pstv = pst[:].bitcast(F16)
                nc.tensor.transpose(pstv[0:128, 0:64], src_fm, id16[0:64, 0:64])
                nc.scalar.activation(rows_b[:, 0:64], pstv[0:128, 0:64], AF.Copy)
                if lhsT_B is not None:
                    if isinstance(lhsT_B, str):
                        tb = transp(xb_ap, F16, "tb")
                    else:
                        ps2 = pp.tile([128, CH], F32, tag="mmq")
                        nc.tensor.matmul(ps2[0:64, 0:128], lhsT_B, xb_ap, start=True, stop=True)
                        xb2 = sp.tile([64, 128], F16, tag="xb")
                        nc.scalar.activation(xb2[:], ps2[0:64, 0:128], AF.Copy)
                        tb = transp(xb2[:], F16, "tb")
                    nc.vector.tensor_copy(rows_b[:, 64:128], tb[:, 0:64])
                if lhsT_B is None and width > 64:
                    nc.vector.memset(rows_b[:, 64:128], 0.0)
                if src_rep is not None:
                    m = sp.tile([128, H], F32, tag="auxm")
                    nc.vector.tensor_mul(m[:], rows_b[:, 0:64], src_rep)
                    nc.vector.tensor_reduce(rows_b[:, src_col:src_col + 1], m[:], mybir.AxisListType.X, OP.add)
                if dst_rep is not None:
                    m2 = sp.tile([128, H], F32, tag="dvm")
                    nc.vector.tensor_mul(m2[:], rows_b[:, 0:64], dst_rep)
                    nc.vector.tensor_reduce(dv[:, b:b + 1], m2[:], mybir.AxisListType.X, OP.add)

                eng = nc.sync if b % 2 == 0 else nc.scalar
                eng.dma_start(cview[b], rows_b[:, 0:width])
                if mid_cb is not None and b == NBLK // 2 - 1:
                    mid_cb()
            return dv

        HB = (NBLK // 2) * 128
        HQ = HB // 2

        def coll_pair(l):
            def fire_a():
                nc.gpsimd.collective_compute(
                    "AllGather", OP.bypass, ins=[cins[l][0:HB]],
                    outs=[couts[l][0:NCORE * HQ]], replica_groups=[list(range(NCORE))])

            def fire_b():
                nc.gpsimd.collective_compute(
                    "AllGather", OP.bypass, ins=[cins[l][HB:NPAD]],
                    outs=[couts[l][NCORE * HQ:TROWS // 2]], replica_groups=[list(range(NCORE))])

            return fire_a, fire_b

        # ================== forward ==================
        stack0 = stkp.tile([128, NPAD], F16, tag="stk")
        xsep = xsp.tile([64, NPAD], F16, tag="xsep")
        nc.sync.dma_start(stack0[0:64, 0:NPAD // 2], xT_in[:, 0:NPAD // 2])
        nc.scalar.dma_start(stack0[0:64, NPAD // 2:], xT_in[:, NPAD // 2:])
        fm_mm(xsep[:], lin1T_s[:], stack0[0:64, :], NPAD, func=AF.Lrelu, bias=lin1b_s[:])
        nc.sync.dma_start(stack0[64:128, :], xsep[:])
        rv = big.tile([128, NBLK], F32, tag="rvals0")
        ca, cb = coll_pair(0)
        build_rows(cins[0], 128, xsep[:], None, w1aT_s[:], attr_s[:], None, 0, dv=rv,
                   mid_cb=ca)
        cb()
        edge_phase(0, couts[0], rv, stack0)
        elu_inplace(stack0, gateb_s[:], NPAD, pre_lhsT=g2T_s[:])
        xcur, xsep_cur = stack0, xsep
        for l in range(2):
            xnew = stkp.tile([128, NPAD], F16, tag="stk")
            xsep_n = xsp.tile([64, NPAD], F16, tag="xsep")
            gru(xcur, xsep_cur, l, xsep_n[:], NPAD)
            nc.sync.dma_start(xnew[64:128, :], xsep_n[:])
            dv = big.tile([128, NBLK], F32, tag=f"rvals{l + 1}")
            ca, cb = coll_pair(l + 1)
            build_rows(cins[l + 1], 64, xsep_n[:], None, None,
                       adst_s[:, l], None, 0, dv=dv, mid_cb=ca)
            cb()
            edge_phase(l + 1, couts[l + 1], dv, xnew)
            elu_inplace(xnew, atomb_s[:, l:l + 1], NPAD, pre_lhsT=atomT_s[:, l])
            xcur, xsep_cur = xnew, xsep_n
        xfin = xsp.tile([64, NPAD], F16, tag="xsep")
        gru(xcur, xsep_cur, 2, xfin[:], NPAD)
        build_rows(routT, 128, xfin[:], None, None, None, None, 0)
        # readout: gather x slots from LOCAL table (resident rrow), on-chip
        # a_src (sres), masked out0 accumulation
        rrow = big.tile([128, GCH, 64], F16, tag="rrow")
        sres = big.tile([128, GCH, 1], F32, tag="sres")
        for b in range(GBLK):
            for (r0, r1) in gcalls[b]:
                cr = r1 - r0
                ch0 = gbases[b] + r0
                g = grp_.tile([128, MAXRG, 128], F16, tag="gr")
                nc.gpsimd.dma_gather(
                    g[:, 0:cr], routT[:], rgidx_s[:, ch0 * 8:(ch0 + cr) * 8],
                    cr * 128, cr * 128, 128, elem_step=128,
                    single_packet=False, queue_num=qctr[0] % 4)
                qctr[0] += 1
                nc.vector.tensor_mul(rrow[:, ch0:ch0 + cr], g[:, 0:cr, 0:64],
                                      rmsk16_s[:, ch0:ch0 + cr].to_broadcast([128, cr, 64]))
                sv = sp.tile([128, MAXRG, 64], F16, tag="svr")
                nc.vector.tensor_mul(sv[:, 0:cr], rrow[:, ch0:ch0 + cr],
                                     msrc_s[:].to_broadcast([128, cr, 64]))
                nc.vector.tensor_reduce(sres[:, ch0:ch0 + cr], sv[:, 0:cr], mybir.AxisListType.X, OP.add)
        ofm = big.tile([64, GPAD], F16, tag="ofm")
        hro = big.tile([64, GPAD], F16, tag="hro")
        mol_stk = big.tile([128, GPAD], F16, tag="mstk")
        for b in range(GBLK):
            ps0 = rp.tile([128, 64], F32, tag="red")
            RGb = int(RG[b])
            for r in range(RGb):
                nc.tensor.matmul(ps0[:], id16[:], rrow[:, gbases[b] + r],
                                 start=(r == 0), stop=(r == RGb - 1))
            s0 = sp.tile([128, 64], F32, tag="hnm")
            nc.scalar.activation(s0[:], ps0[:], AF.Copy)
            t0 = transp(s0[:], F32, "th")
            nc.scalar.activation(ofm[:, b * 128:(b + 1) * 128], t0[0:64, 0:128], AF.Relu)
        wtil_ps = rp.tile([64, 1], F32, tag="red")
        nc.tensor.matmul(wtil_ps[:], mol_lin_s[:], mdst_s[:], start=True, stop=True)
        wtil = cpool.tile([64, 1], F16)
        nc.vector.tensor_copy(wtil[:], wtil_ps[:])
        for t in range(3):
            ddp = rp.tile([1, GPAD], F32, tag="red")
            nc.tensor.matmul(ddp[:], wtil[:], ofm[:], start=True, stop=True)
            dds = s1.tile([1, GPAD], F32, tag="dds")
            nc.vector.tensor_copy(dds[:], ddp[:])
            for b in range(GBLK):
                ddb = transp(dds[:, b * 128:(b + 1) * 128], F32, "ddb")
                RGb = int(RG[b])
                gb0 = gbases[b]
                pred = rp.tile([128, 64], F32, tag="red")
                ddb001 = sp.tile([128, 1], F32, tag="ddb001")
                nc.vector.tensor_scalar(ddb001[:], ddb[:, 0:1], NS, None, OP.mult)
                e1 = sp.tile([128, GCH, 1], F32, tag="lgro")
                nc.scalar.activation(e1[:, gb0:gb0 + RGb], sres[:, gb0:gb0 + RGb],
                                     AF.Exp, bias=ddb[:, 0:1])
                e2 = sp.tile([128, GCH, 1], F32, tag="lgro2")
                nc.scalar.activation(e2[:, gb0:gb0 + RGb], sres[:, gb0:gb0 + RGb],
                                     AF.Exp, scale=NS, bias=ddb001[:, 0:1])
                nc.vector.tensor_max(e1[:, gb0:gb0 + RGb], e1[:, gb0:gb0 + RGb], e2[:, gb0:gb0 + RGb])
                p16 = sp.tile([128, GCH, 1], F16, tag="p16r")
                dsum = sp.tile([128, 1], F32, tag="dsum")
                nc.vector.scalar_tensor_tensor(
                    p16[:, gb0:gb0 + RGb], e1[:, gb0:gb0 + RGb], 1.0,
                    rmsk_s[:, gb0:gb0 + RGb], OP.mult, OP.mult, accum_out=dsum[:])
                msg = sp.tile([128, GCH, 64], F16, tag="msgr")
                nc.vector.tensor_mul(msg[:, gb0:gb0 + RGb], rrow[:, gb0:gb0 + RGb, 0:64],
                                     p16[:, gb0:gb0 + RGb].to_broadcast([128, RGb, 64]))
                for r in range(RGb):
                    nc.tensor.matmul(pred[:, 0:64], id16[:], msg[:, gb0 + r],
                                     start=(r == 0), stop=(r == RGb - 1))
                rec = sp.tile([128, 1], F32, tag="rec")
                nc.vector.tensor_scalar(rec[:], dsum[:], 1e-16, None, OP.add)
                nc.vector.reciprocal(rec[:], rec[:])
                hnm = sp.tile([128, 64], F32, tag="hnm")
                nc.scalar.activation(hnm[:], pred[:, 0:64], AF.Copy, scale=rec[:])
                th = transp(hnm[:], F32, "th")
                nc.scalar.activation(hro[:, b * 128:(b + 1) * 128], th[0:64, 0:128], AF.Copy)
            elu_inplace(hro, molb_s[:], GPAD, pre_lhsT=molT_s[:])
            nc.vector.tensor_copy(mol_stk[0:64, :], hro[:])
            nc.vector.tensor_copy(mol_stk[64:128, :], ofm[:])
            onew = s1.tile([64, GPAD], F16, tag="onew")
            gru(mol_stk, ofm, 3, onew[:], GPAD)
            nc.vector.tensor_copy(ofm[:], onew[:])
        emb = sp.tile([64, GPAD], F32, tag="emb")
        fm_mm(emb[:], lin2T_s[:], ofm[:], GPAD, bias=lin2b_s[:])
        nemb = sp.tile([64, GPAD], F16, tag="nemb")
        for b in range(GBLK):
            gm = transp(emb[:, b * 128:(b + 1) * 128], F32, "gm")
            mu = sp.tile([128, 1], F32, tag="mu")
            nc.vector.tensor_reduce(mu[:], gm[:, 0:64], mybir.AxisListType.X, OP.add)
            nc.vector.tensor_scalar(mu[:], mu[:], 1.0 / 64, None, OP.mult)
            xc = sp.tile([128, 64], F32, tag="xc")
            nc.vector.tensor_scalar(xc[:], gm[:, 0:64], mu[:], None, OP.subtract)
            sq = sp.tile([128, 64], F32, tag="sq")
            nc.scalar.activation(sq[:], xc[:], AF.Square)
            var = sp.tile([128, 1], F32, tag="var")
            nc.vector.tensor_reduce(var[:], sq[:], mybir.AxisListType.X, OP.add)
            nc.vector.tensor_scalar(var[:], var[:], 1.0 / 64, None, OP.mult)
            nc.vector.tensor_scalar(var[:], var[:], 1e-5, None, OP.add)
            nc.scalar.activation(var[:], var[:], AF.Sqrt)
            nc.vector.reciprocal(var[:], var[:])
            nc.scalar.activation(xc[:], xc[:], AF.Copy, scale=var[:])
            nc.vector.tensor_mul(xc[:], xc[:], lng_s[:, 0:64])
            nc.vector.tensor_add(xc[:], xc[:], lnb_s[:, 0:64])
            tb = transp(xc[:], F16, "tb2")
            nc.vector.tensor_copy(nemb[:, b * 128:(b + 1) * 128], tb[0:64, 0:128])
        m1 = sp.tile([64, GPAD], F16, tag="m1")
        fm_mm(m1[:], h1T_s[:], nemb[:], GPAD, func=AF.Relu, bias=h1b_s[:])
        m2 = sp.tile([64, GPAD], F16, tag="m2")
        fm_mm(m2[:], h2T_s[:], m1[:], GPAD, func=AF.Relu, bias=h2b_s[:])
        yps = rp.tile([1, GPAD], F32, tag="red")
        nc.tensor.matmul(yps[:], h3T_s[:], m2[:], start=True, stop=True)
        ysb = s1.tile([1, GPAD], F32, tag="ysb")
        nc.scalar.activation(ysb[:], yps[:], AF.Identity, bias=h3b_s[:])
        nc.sync.dma_start(y_out[:], ysb[:])
        ctx.close()
    ctx2.__exit__(None, None, None)
    nc.finalize()
    return nc


_CACHE = {}


def kernel(**inputs):
    x = np.asarray(inputs["x"], np.float32)
    edge_attr = np.asarray(inputs["edge_attr"], np.float32)
    ei = np.asarray(inputs["edge_index"])
    batch = np.asarray(inputs["batch"])
    if "k" not in _CACHE:
        plan = build_plan(ei, batch)
        nc = build_nc(plan)
        _CACHE["k"] = (plan, nc)
    plan, nc = _CACHE["k"]
    NPAD = plan["NPAD"]

    gw = np.zeros((128, 4, 128), np.float16)
    gn = np.zeros((128, 4, 128), np.float16)
    gbx_rz = np.zeros((64, 8), np.float32)
    gbh_rz = np.zeros((64, 8), np.float32)
    gbx_n = np.zeros((H, 4), np.float32)
    gbh_n = np.zeros((H, 4), np.float32)
    packs = [
        (inputs["gru0_wx"], inputs["gru0_wh"], inputs["gru0_bx"], inputs["gru0_bh"]),
        (inputs["atom_gru_wx"][0], inputs["atom_gru_wh"][0], inputs["atom_gru_bx"][0], inputs["atom_gru_bh"][0]),
        (inputs["atom_gru_wx"][1], inputs["atom_gru_wh"][1], inputs["atom_gru_bx"][1], inputs["atom_gru_bh"][1]),
        (inputs["mol_gru_wx"], inputs["mol_gru_wh"], inputs["mol_gru_bx"], inputs["mol_gru_bh"]),
    ]
    for i, (wx, wh, bx, bh) in enumerate(packs):
        wx = np.asarray(wx, np.float32); wh = np.asarray(wh, np.float32)
        bx = np.asarray(bx, np.float32); bh = np.asarray(bh, np.float32)
        gw[0:64, i, 0:64] = wx[0:64].T; gw[64:128, i, 0:64] = wh[0:64].T
        gw[0:64, i, 64:128] = wx[64:128].T; gw[64:128, i, 64:128] = wh[64:128].T
        gn[0:64, i, 0:64] = wx[128:192].T; gn[64:128, i, 64:128] = wh[128:192].T
        gbx_rz[:, 2 * i] = bx[0:64]; gbx_rz[:, 2 * i + 1] = bx[64:128]
        gbh_rz[:, 2 * i] = bh[0:64]; gbh_rz[:, 2 * i + 1] = bh[64:128]
        gbx_n[:, i] = bx[128:192]; gbh_n[:, i] = bh[128:192]

    glw = np.asarray(inputs["gate_lin1_w"], np.float32)
    rep16 = lambda v: np.tile(np.asarray(v, np.float16).reshape(1, -1), (128, 1))
    rep32 = lambda v: np.tile(np.asarray(v, np.float32).reshape(1, -1), (128, 1))
    a = lambda k: np.asarray(inputs[k], np.float32)
    f16 = lambda v: np.asarray(v, np.float16)
    wts = dict(
        lin1T=f16(a("lin1_w").T), lin1_b=a("lin1_b").reshape(H, 1),
        w1aT=f16(glw[:, 0:64].T),
        w1bT=f16(glw[:, 64:80].T),
        attl_rep=rep16(inputs["gate_att_l"]).reshape(128, 1, H),
        attr_rep=rep16(inputs["gate_att_r"]),
        g2T=f16(a("gate_lin2_w").T), gate_b=a("gate_bias").reshape(H, 1),
        atomT=f16(np.stack([a("atom_lin_w")[l].T for l in range(2)], 1)),
        asrc_rep=np.stack([rep16(a("atom_lin_w")[l].T @ a("atom_att_src")[l]) for l in range(2)], 1),
        adst_rep=np.stack([rep16(a("atom_lin_w")[l].T @ a("atom_att_dst")[l]) for l in range(2)], 1),
        atom_b=a("atom_bias").T.copy(),
        molT=f16(a("mol_lin_w").T), mol_lin=a("mol_lin_w").copy(),
        matt_src_rep=rep16(a("mol_lin_w").T @ a("mol_att_src")).reshape(128, 1, H),
        matt_dst=a("mol_att_dst").reshape(H, 1),
        mol_b=a("mol_bias").reshape(H, 1),
        gruW=gw, gruN=gn, gbx_rz=gbx_rz, gbh_rz=gbh_rz, gbx_n=gbx_n, gbh_n=gbh_n,
        lin2T=f16(a("lin2_w").T), lin2_b=a("lin2_b").reshape(H, 1),
        lng_rep=rep32(inputs["ln_g"]), lnb_rep=rep32(inputs["ln_b"]),
        h1T=f16(a("h1_w").T), h1_b=a("h1_b").reshape(H, 1),
        h2T=f16(a("h2_w").T), h2_b=a("h2_b").reshape(H, 1),
        h3T=f16(a("h3_w").T), h3_b=a("h3_b").reshape(1, 1),
    )
    in_maps = []
    for c in range(NCORE):
        pc = plan["cores"][c]
        rc = plan["rcores"][c]
        lo, hi = int(plan["bounds"][c]), int(plan["bounds"][c + 1])
        xT = np.zeros((H, NPAD), np.float16)
        xT[:, 0:hi - lo] = x[lo:hi][plan["perms"][c]].T
        eaT = np.zeros((16, plan["S"]), np.float16)
        val = pc["esel"] >= 0
        eaT[:, val] = edge_attr[pc["esel"][val]].T.astype(np.float16)
        im = dict(xT_in=xT, gidx=pc["gidx"], par=pc["par"], msk=pc["msk"], eaT=eaT,
                  rgidx=rc["rgidx"], rmsk=rc["rmsk"], rmsk16=rc["rmsk16"], **wts)
        in_maps.append(im)
    res = run_bass_kernel_spmd(nc, in_maps, core_ids=list(range(NCORE)))
    if getattr(res, "exec_time_ns", None):
        print(f"HW exec time: {res.exec_time_ns} ns", flush=True)
    y = np.zeros(B, np.float32)
    for c in range(NCORE):
        ys = res.results[c]["y_out"].reshape(GPAD)[0:GPC]
        y[c * GPC + plan["gperms"][c]] = ys
    return y.reshape(B, 1)
